# revision 21
# baseline (speedup 1.0000x reference)
"""Trainium2 Bass kernel for nn_CoAttention.

Sharding: data-parallel over batch. B=16 across 8 cores -> 2 batches/core.
All weights replicated. No collectives.

Attention (per local batch b), all fp16 operands / fp32 PSUM:
  h_sT = tanh(W_ref @ src_b.T + b_ref)          [H, S]
  h_sN = h_sT.T (PE transpose)
  eT   = exp(h_r @ h_s.T)  with fused Ds accum  [R, S]
  eN   = eT.T (PE transpose), Dr by free reduce
  a_sT = eT/Ds   eS = eN/Dr
  c_sT = a_sT.T @ h_rN                           [S, H]
  c_rT = [h_sN | c_sT].T @ eS                    [2H, R]
  xg_d = W_ihT_d.T @ [c_rT; h_rT] + b            gate pre-acts, all t

LSTM: time-chunked scan with warmup. Each direction's 512 steps are split
into J chunks of CL=512/J processed as extra COLUMNS of the same
instructions; each chunk warms up for W steps from zero state (gate
pre-acts = -30 in the pad region force sigmoid=0 => state stays exactly 0
until real steps begin; forget-gate products damp the h0/c0 mismatch by
~e^-0.6W, validated < 3e-7 on the real data for W>=48). The backward
direction shares the forward layout: its chunks scan the buffer with a
descending per-step base, so all access patterns are positive-stride.

Scan layouts (J slots, NB = R + W blocks per xg buffer):
  xgbuf[d]  [128, 8 * 2NB]   col = g*2NB + blk*2 + b     (g = 8 gate tiles,
             permuted gate order i,f,o,g~; blk = time block; b = local batch)
  ps (PSUM) [128, 16J]       col = gt*4J + k*2J + j*2 + b (gt = gate type,
             k = hidden chunk, j = chunk slot) -- each (gt,k) region is a
             contiguous 2J-col matmul target
  outslab[d][128, (STEPS+1)*4J] fp16, block s = h after step s-1;
             col-in-block = k*2J + j*2 + b; doubles as the h feedback buffer
Per step: 8x [xg inject via identity matmul (start=True) + 2 W_hh matmuls],
one sigmoid over all gates (tanh(x) = 2*sigmoid(2x)-1, with the 2x folded
into the host-side g~ weight scaling), 3 DVE ops for the cell update, one
tanh, one DVE mul writing h directly into outslab.
"""

import numpy as np
import ml_dtypes

import concourse.bass as bass
import concourse.mybir as mybir
import concourse.tile as tile
from concourse import bacc
from concourse import bass_utils

BF16 = ml_dtypes.bfloat16
FP16 = np.float16

B, S, R, H = 16, 1024, 512, 512
HD = H // 2          # 256
G = 4 * HD           # 1024
DIN = 3 * H          # 1536
N_CORES = 8
BLOC = B // N_CORES  # 2

# --- chunked-scan parameters ---
J = 16               # chunks per direction (also column slots per step)
W = 24               # warmup steps per chunk
CL = R // J          # chunk length
STEPS = W + CL       # sequential steps per direction
NB = R + W           # time blocks in the xg buffer (incl. warmup pad)

INJECTS_FIRST = False    # True corrupts PSUM accumulation on HW (groups must be
                         # emitted contiguously: inject then its W_hh matmuls)
ALIAS_CST = True         # write c_sT into the spent srcT region

# Gate-type permutation (host side): blocks reordered i,f,o,g so sigmoid
# gates are contiguous; g~ block additionally scaled x2 for the
# tanh(x) = 2*sigmoid(2x)-1 rewrite.
_GPERM = np.r_[0:512, 768:1024, 512:768]

F32 = mybir.dt.float32
F16 = mybir.dt.float16
BF = mybir.dt.bfloat16
AF = mybir.ActivationFunctionType
ALU = mybir.AluOpType

_CACHE = {}


def _build_nc():
    nc = bacc.Bacc("TRN2", target_bir_lowering=False, debug=False,
                   num_devices=N_CORES)

    # ---- DRAM I/O (all host-prepped [128, F] SBUF images) ----
    d_srcT = nc.dram_tensor("srcT", [128, BLOC * 4 * S], F16, kind="ExternalInput")
    d_hrT = nc.dram_tensor("hrT", [128, BLOC * 4 * R], F16, kind="ExternalInput")
    d_hrN = nc.dram_tensor("hrN", [128, BLOC * 4 * H], F16, kind="ExternalInput")
    d_wrefT = nc.dram_tensor("wrefT", [128, 4 * H], F16, kind="ExternalInput")
    d_brefT = nc.dram_tensor("brefT", [128, 4], F32, kind="ExternalInput")
    d_wih = {d: nc.dram_tensor(f"wihT_{d}", [128, 12 * G], F16, kind="ExternalInput")
             for d in "fb"}
    d_whh = {d: nc.dram_tensor(f"whhT_{d}", [128, 2 * G], F16, kind="ExternalInput")
             for d in "fb"}
    d_bg = {d: nc.dram_tensor(f"bgT_{d}", [128, 8], F32, kind="ExternalInput")
            for d in "fb"}
    d_id16 = nc.dram_tensor("id16", [128, 128], F16, kind="ExternalInput")
    d_idbf = nc.dram_tensor("idbf", [128, 128], BF, kind="ExternalInput")
    d_out = {d: nc.dram_tensor(f"out_{d}", [128, (STEPS + 1) * 4 * J], F16,
                               kind="ExternalOutput")
             for d in "fb"}

    with tile.TileContext(nc) as tc, \
         tc.tile_pool(name="wp", bufs=1) as wp, \
         tc.tile_pool(name="ap", bufs=1) as ap, \
         tc.tile_pool(name="scansb", bufs=1) as scansb, \
         tc.tile_pool(name="pp", bufs=2, space="PSUM") as pp, \
         tc.tile_pool(name="psc", bufs=2, space="PSUM") as psc:

        # ---- persistent loads ----
        def load(dram, shape, dt):
            t = wp.tile(shape, dt, tag=dram.name, name=dram.name)
            nc.sync.dma_start(t[:], dram[:])
            return t

        # order matters: srcT/wref are needed first (attention stage 1),
        # the 7MB of LSTM weights only at xg time much later
        srcT = load(d_srcT, [128, BLOC * 4 * S], F16)
        wrefT = load(d_wrefT, [128, 4 * H], F16)
        brefT = load(d_brefT, [128, 4], F32)
        id16 = load(d_id16, [128, 128], F16)
        idbf = load(d_idbf, [128, 128], BF)
        hrT = load(d_hrT, [128, BLOC * 4 * R], F16)
        hrN = load(d_hrN, [128, BLOC * 4 * H], F16)
        whh = {d: load(d_whh[d], [128, 2 * G], F16) for d in "fb"}
        bg = {d: load(d_bg[d], [128, 8], F32) for d in "fb"}
        wih = {d: load(d_wih[d], [128, 12 * G], F16) for d in "fb"}

        xg = {d: wp.tile([128, 8 * 2 * NB], F16, tag=f"xg_{d}", name=f"xg_{d}")
              for d in "fb"}
        outb = {d: wp.tile([128, (STEPS + 1) * 4 * J], F16, tag=f"outsb_{d}",
                           name=f"outsb_{d}") for d in "fb"}

        # warmup pads: forward pads blocks [0, W), backward pads [R, NB)
        for g in range(8):
            nc.vector.memset(xg["f"][:, g * 2 * NB: g * 2 * NB + 2 * W], -30.0)
            nc.vector.memset(xg["b"][:, g * 2 * NB + 2 * R: (g + 1) * 2 * NB], -30.0)

        # ---- attention + xg, per local batch ----
        for b in range(BLOC):
            hrT_b = hrT[:, b * 4 * R:(b + 1) * 4 * R]
            hrN_b = hrN[:, b * 4 * H:(b + 1) * 4 * H]

            srcT_b = srcT[:, b * 4 * S:(b + 1) * 4 * S]

            # 1) h_sT [4 Hout-tiles x S]
            hsT = ap.tile([128, 4 * S], F16, tag="tagB")
            for m in range(4):
                for sc in range(2):
                    ps = pp.tile([128, 512], F32, tag="mm")
                    for k in range(4):
                        nc.tensor.matmul(
                            ps[:],
                            wrefT[:, k * H + m * 128: k * H + (m + 1) * 128],
                            srcT_b[:, k * S + sc * 512: k * S + sc * 512 + 512],
                            start=(k == 0), stop=(k == 3))
                    nc.scalar.activation(
                        hsT[:, m * S + sc * 512: m * S + sc * 512 + 512],
                        ps[:], AF.Tanh, bias=brefT[:, m:m + 1])

            # 2) h_sN [8 S-tiles x H] = transpose(h_sT) via DMA xbar
            hsN = ap.tile([128, 8 * H], F16, tag="tagC")
            hsNv = hsN.rearrange("p (st h) -> p st h", st=8)
            for hc in range(4):
                nc.sync.dma_start_transpose(
                    hsNv[:, :, hc * 128:(hc + 1) * 128],
                    hsT[:, hc * S:(hc + 1) * S])

            # 3) eT [4 R-tiles x S] = exp(l.T), Ds partials fused into accum_out
            eT = ap.tile([128, 4 * S], BF, tag="tagD")
            ds2 = ap.tile([128, 8], F32, tag="ds2")
            for rt in range(4):
                for sc in range(2):
                    ps = pp.tile([128, 512], F32, tag="mm")
                    for k in range(4):
                        nc.tensor.matmul(
                            ps[:],
                            hrT_b[:, k * R + rt * 128: k * R + (rt + 1) * 128],
                            hsT[:, k * S + sc * 512: k * S + sc * 512 + 512],
                            start=(k == 0), stop=(k == 3))
                    nc.scalar.activation(
                        eT[:, rt * S + sc * 512: rt * S + sc * 512 + 512],
                        ps[:], AF.Exp,
                        accum_out=ds2[:, rt * 2 + sc: rt * 2 + sc + 1])

            # 4) eN [8 S-tiles x R] = transpose(eT) via DMA xbar
            eN = ap.tile([128, 8 * R], BF, tag="tagE")
            eNv = eN.rearrange("p (st r) -> p st r", st=8)
            for rc in range(4):
                nc.sync.dma_start_transpose(
                    eNv[:, :, rc * 128:(rc + 1) * 128],
                    eT[:, rc * S:(rc + 1) * S])

            # 5) softmax denominators -> scaled copies (fp16)
            dsum = ap.tile([128, 4], F32, tag="dsum")
            for rt in range(4):
                nc.vector.tensor_add(dsum[:, rt:rt + 1], ds2[:, 2 * rt:2 * rt + 1],
                                     ds2[:, 2 * rt + 1:2 * rt + 2])
            invDs = ap.tile([128, 4], F32, tag="invDs")
            nc.vector.reciprocal(invDs[:], dsum[:])
            drsum = ap.tile([128, 8], F32, tag="drsum")
            for st in range(8):
                nc.vector.tensor_reduce(
                    drsum[:, st:st + 1], eN[:, st * R:(st + 1) * R],
                    mybir.AxisListType.X, ALU.add)
            invDr = ap.tile([128, 8], F32, tag="invDr")
            nc.vector.reciprocal(invDr[:], drsum[:])

            asT = ap.tile([128, 4 * S], F16, tag="tagF")
            for rt in range(4):
                nc.vector.tensor_scalar_mul(
                    asT[:, rt * S:(rt + 1) * S], eT[:, rt * S:(rt + 1) * S],
                    invDs[:, rt:rt + 1])
            eS = ap.tile([128, 8 * R], F16, tag="tagG")
            for st in range(8):
                nc.vector.tensor_scalar_mul(
                    eS[:, st * R:(st + 1) * R], eN[:, st * R:(st + 1) * R],
                    invDr[:, st:st + 1])

            # 6) c_sT [8 S-tiles x H]  (overwrites this batch's spent srcT)
            if ALIAS_CST:
                csT = srcT[:, b * 4 * S:(b + 1) * 4 * S]
            else:
                csT = ap.tile([128, 8 * H], F16, tag="tagA")
            for st in range(8):
                ps = pp.tile([128, 512], F32, tag="mm")
                for k in range(4):
                    nc.tensor.matmul(
                        ps[:],
                        asT[:, k * S + st * 128: k * S + st * 128 + 128],
                        hrN_b[:, k * H: (k + 1) * H],
                        start=(k == 0), stop=(k == 3))
                nc.vector.tensor_copy(csT[:, st * H:(st + 1) * H], ps[:])

            # 7) c_rT [8 2H-tiles x R]  (reuses hsT slot after last hsT read)
            crT = ap.tile([128, 8 * R], F16, tag="tagB2")
            for m in range(8):
                ps = pp.tile([128, 512], F32, tag="mm")
                for k in range(8):
                    if m < 4:
                        lhsT = hsN[:, k * H + m * 128: k * H + m * 128 + 128]
                    else:
                        lhsT = csT[:, k * H + (m - 4) * 128: k * H + (m - 4) * 128 + 128]
                    nc.tensor.matmul(ps[:], lhsT, eS[:, k * R:(k + 1) * R],
                                     start=(k == 0), stop=(k == 7))
                nc.vector.tensor_copy(crT[:, m * R:(m + 1) * R], ps[:])

            # 8) xg per direction, strided into the scan's blk-major layout
            #    forward real blocks start at W, backward at 0
            for d in "fb":
                off = 2 * W if d == "f" else 0
                for g in range(8):
                    ps = pp.tile([128, 512], F32, tag="mm")
                    for k in range(12):
                        if k < 8:
                            rhs = crT[:, k * R:(k + 1) * R]
                        else:
                            rhs = hrT_b[:, (k - 8) * R:(k - 7) * R]
                        nc.tensor.matmul(
                            ps[:],
                            wih[d][:, k * G + g * 128: k * G + (g + 1) * 128],
                            rhs, start=(k == 0), stop=(k == 11))
                    dst = xg[d][:, g * 2 * NB + off + b: g * 2 * NB + off + 2 * R: 2]
                    nc.vector.tensor_scalar_add(dst, ps[:], bg[d][:, g:g + 1])

        # ---- chunked LSTM scan ----
        cst = {d: wp.tile([128, 4 * J], F32, tag=f"c_{d}", name=f"c_{d}")
               for d in "fb"}
        for d in "fb":
            nc.vector.memset(cst[d][:], 0.0)
            nc.vector.memset(outb[d][:, 0:4 * J], 0.0)

        # blk-major views of xg for the inject matmuls: [128, g, NB, 2]
        xgv = {d: xg[d].rearrange("p (g blk b) -> p g blk b", g=8, b=2)
               for d in "fb"}

        for t in range(STEPS):
            for d in "fb":
                # first block index read this step (per slot j: blk0 + j*CL)
                blk0 = t if d == "f" else (CL - 1 + W - t)
                ps = psc.tile([128, 16 * J], F32, tag=f"scps_{d}")
                hprev = outb[d][:, t * 4 * J:(t + 1) * 4 * J]
                if INJECTS_FIRST:
                    # xg injects don't depend on h -> issue them all first so
                    # the PE works while the previous step's h is in flight
                    for g in range(8):
                        nc.tensor.matmul(
                            ps[:, g * 2 * J:(g + 1) * 2 * J], id16[:],
                            xgv[d][:, g, blk0: blk0 + (J - 1) * CL + 1: CL, :],
                            start=True, stop=False)
                for g in range(8):
                    reg = ps[:, g * 2 * J:(g + 1) * 2 * J]
                    if not INJECTS_FIRST:
                        nc.tensor.matmul(
                            reg, id16[:],
                            xgv[d][:, g, blk0: blk0 + (J - 1) * CL + 1: CL, :],
                            start=True, stop=False)
                    for k in range(2):
                        nc.tensor.matmul(
                            reg,
                            whh[d][:, k * G + g * 128: k * G + (g + 1) * 128],
                            hprev[:, k * 2 * J:(k + 1) * 2 * J],
                            start=False, stop=(k == 1))
                acts = scansb.tile([128, 16 * J], F32, tag=f"acts_{d}")
                nc.scalar.activation(acts[:], ps[:], AF.Sigmoid)
                si = acts[:, 0:4 * J]
                sf = acts[:, 4 * J:8 * J]
                so = acts[:, 8 * J:12 * J]
                sg = acts[:, 12 * J:16 * J]
                cprod = scansb.tile([128, 4 * J], F32, tag=f"cprod_{d}")
                nc.vector.tensor_mul(cprod[:], cst[d][:], sf)
                t1h = scansb.tile([128, 4 * J], F32, tag=f"t1h_{d}")
                nc.vector.scalar_tensor_tensor(
                    t1h[:], sg, 0.5, si, ALU.subtract, ALU.mult)
                nc.vector.scalar_tensor_tensor(
                    cst[d][:], t1h[:], 2.0, cprod[:], ALU.mult, ALU.add)
                th = scansb.tile([128, 4 * J], F16, tag=f"th_{d}")
                nc.scalar.activation(th[:], cst[d][:], AF.Tanh)
                nc.vector.tensor_mul(
                    outb[d][:, (t + 1) * 4 * J:(t + 2) * 4 * J], th[:], so)

        for d in "fb":
            nc.sync.dma_start(d_out[d][:], outb[d][:])

    nc.compile()
    return nc


def _img_kmaj(x, p=128):
    """[K, F] -> [128, (K/128)*F] k-tile image."""
    k, f = x.shape
    return np.ascontiguousarray(
        x.reshape(k // p, p, f).transpose(1, 0, 2).reshape(p, (k // p) * f))


def _prep_core(core, inp):
    gb = [BLOC * core + i for i in range(BLOC)]
    src = np.asarray(inp["src_memory_bank"])   # [S, B, H]
    ref = np.asarray(inp["ref_memory_bank"])   # [R, B, H]

    def cat(imgs):
        return np.concatenate(imgs, axis=1)

    # g~ pre-act scale for tanh(x) = 2*sigmoid(2x)-1 (last 256 permuted rows)
    gscale = np.ones((G, 1), np.float64)
    gscale[768:] = 2.0

    m = {}
    m["srcT"] = cat([_img_kmaj(src[:, b, :].T.astype(FP16)) for b in gb])
    m["hrT"] = cat([_img_kmaj(ref[:, b, :].T.astype(FP16)) for b in gb])
    m["hrN"] = cat([_img_kmaj(ref[:, b, :].astype(FP16)) for b in gb])
    m["wrefT"] = _img_kmaj(np.asarray(inp["W_ref"]).T.astype(FP16))
    m["brefT"] = np.ascontiguousarray(
        np.asarray(inp["b_ref"]).astype(np.float32).reshape(4, 128).T)
    for d, sfx in (("f", "_f"), ("b", "_b")):
        wih = np.asarray(inp[f"W_ih{sfx}"], dtype=np.float64)[_GPERM] * gscale
        whh = np.asarray(inp[f"W_hh{sfx}"], dtype=np.float64)[_GPERM] * gscale
        m[f"wihT_{d}"] = _img_kmaj(wih.T.astype(FP16))
        m[f"whhT_{d}"] = _img_kmaj(whh.T.astype(FP16))
        bsum = ((np.asarray(inp[f"b_ih{sfx}"], dtype=np.float64)
                 + np.asarray(inp[f"b_hh{sfx}"], dtype=np.float64))[_GPERM]
                * gscale[:, 0])
        m[f"bgT_{d}"] = np.ascontiguousarray(
            bsum.astype(np.float32).reshape(8, 128).T)
    m["id16"] = np.eye(128, dtype=FP16)
    m["idbf"] = np.eye(128, dtype=BF16)
    return m


def _decode(res_list):
    """results -> [R, B, H] fp32"""
    out = np.zeros((R, B, H), dtype=np.float32)
    for c in range(N_CORES):
        for d, off in (("f", 0), ("b", HD)):
            img = np.asarray(res_list[c][f"out_{d}"])          # [128, (STEPS+1)*4J]
            x = img.reshape(128, STEPS + 1, 2, J, 2)           # p, s, k, j, b
            x = x[:, 1 + W: 1 + W + CL]                        # p, tau, k, j, b
            if d == "b":
                x = x[:, ::-1]                                 # tau' -> CL-1-tau'
            # out[j*CL + tau, b, k*128 + p]
            arr = x.transpose(3, 1, 4, 2, 0)                   # j, tau, b, k, p
            arr = np.ascontiguousarray(arr, dtype=np.float32).reshape(R, BLOC, HD)
            out[:, BLOC * c:BLOC * (c + 1), off:off + HD] = arr
    return out


def kernel(**inputs):
    if "nc" not in _CACHE:
        _CACHE["nc"] = _build_nc()
    nc = _CACHE["nc"]
    in_maps = [_prep_core(c, inputs) for c in range(N_CORES)]
    res = bass_utils.run_bass_kernel_spmd(nc, in_maps,
                                          core_ids=list(range(N_CORES)))
    return _decode(res.results)


# revision 25
# speedup vs baseline: 1.0067x; 1.0067x over previous
"""Trainium2 Bass kernel for nn_CoAttention.

Sharding: data-parallel over batch. B=16 across 8 cores -> 2 batches/core.
All weights replicated. No collectives.

Attention (per local batch b), all fp16 operands / fp32 PSUM:
  h_sT = tanh(W_ref @ src_b.T + b_ref)          [H, S]
  h_sN = h_sT.T (PE transpose)
  eT   = exp(h_r @ h_s.T)  with fused Ds accum  [R, S]
  eN   = eT.T (PE transpose), Dr by free reduce
  a_sT = eT/Ds   eS = eN/Dr
  c_sT = a_sT.T @ h_rN                           [S, H]
  c_rT = [h_sN | c_sT].T @ eS                    [2H, R]
  xg_d = W_ihT_d.T @ [c_rT; h_rT] + b            gate pre-acts, all t

LSTM: time-chunked scan with warmup. Each direction's 512 steps are split
into J chunks of CL=512/J processed as extra COLUMNS of the same
instructions; each chunk warms up for W steps from zero state (gate
pre-acts = -30 in the pad region force sigmoid=0 => state stays exactly 0
until real steps begin; forget-gate products damp the h0/c0 mismatch by
~e^-0.6W, validated < 3e-7 on the real data for W>=48). The backward
direction shares the forward layout: its chunks scan the buffer with a
descending per-step base, so all access patterns are positive-stride.

Scan layouts (J slots, NB = R + W blocks per xg buffer):
  xgbuf[d]  [128, 8 * 2NB]   col = g*2NB + blk*2 + b     (g = 8 gate tiles,
             permuted gate order i,f,o,g~; blk = time block; b = local batch)
  ps (PSUM) [128, 16J]       col = gt*4J + k*2J + j*2 + b (gt = gate type,
             k = hidden chunk, j = chunk slot) -- each (gt,k) region is a
             contiguous 2J-col matmul target
  outslab[d][128, (STEPS+1)*4J] fp16, block s = h after step s-1;
             col-in-block = k*2J + j*2 + b; doubles as the h feedback buffer
Per step: 8x [xg inject via identity matmul (start=True) + 2 W_hh matmuls],
one sigmoid over all gates (tanh(x) = 2*sigmoid(2x)-1, with the 2x folded
into the host-side g~ weight scaling), 3 DVE ops for the cell update, one
tanh, one DVE mul writing h directly into outslab.
"""

import numpy as np
import ml_dtypes

import concourse.bass as bass
import concourse.mybir as mybir
import concourse.tile as tile
from concourse import bacc
from concourse import bass_utils

BF16 = ml_dtypes.bfloat16
FP16 = np.float16

B, S, R, H = 16, 1024, 512, 512
HD = H // 2          # 256
G = 4 * HD           # 1024
DIN = 3 * H          # 1536
N_CORES = 8
BLOC = B // N_CORES  # 2

# --- chunked-scan parameters ---
J = 16               # chunks per direction (also column slots per step)
W = 24               # warmup steps per chunk
CL = R // J          # chunk length
STEPS = W + CL       # sequential steps per direction
NB = R + W           # time blocks in the xg buffer (incl. warmup pad)

INJECTS_FIRST = False    # True corrupts PSUM accumulation on HW (groups must be
                         # emitted contiguously: inject then its W_hh matmuls)
ALIAS_CST = True         # write c_sT into the spent srcT region

# Gate-type permutation (host side): blocks reordered i,f,o,g so sigmoid
# gates are contiguous; g~ block additionally scaled x2 for the
# tanh(x) = 2*sigmoid(2x)-1 rewrite.
_GPERM = np.r_[0:512, 768:1024, 512:768]

F32 = mybir.dt.float32
F16 = mybir.dt.float16
BF = mybir.dt.bfloat16
AF = mybir.ActivationFunctionType
ALU = mybir.AluOpType

_CACHE = {}


def _build_nc():
    nc = bacc.Bacc("TRN2", target_bir_lowering=False, debug=False,
                   num_devices=N_CORES)

    # ---- DRAM I/O (all host-prepped [128, F] SBUF images) ----
    d_srcT = nc.dram_tensor("srcT", [128, BLOC * 4 * S], F16, kind="ExternalInput")
    d_hrT = nc.dram_tensor("hrT", [128, BLOC * 4 * R], F16, kind="ExternalInput")
    d_hrN = nc.dram_tensor("hrN", [128, BLOC * 4 * H], F16, kind="ExternalInput")
    d_wrefT = nc.dram_tensor("wrefT", [128, 4 * H], F16, kind="ExternalInput")
    d_brefT = nc.dram_tensor("brefT", [128, 4], F32, kind="ExternalInput")
    d_wih = {d: nc.dram_tensor(f"wihT_{d}", [128, 12 * G], F16, kind="ExternalInput")
             for d in "fb"}
    d_whh = {d: nc.dram_tensor(f"whhT_{d}", [128, 2 * G], F16, kind="ExternalInput")
             for d in "fb"}
    d_bg = {d: nc.dram_tensor(f"bgT_{d}", [128, 8], F32, kind="ExternalInput")
            for d in "fb"}
    d_id16 = nc.dram_tensor("id16", [128, 128], F16, kind="ExternalInput")
    d_idbf = nc.dram_tensor("idbf", [128, 128], BF, kind="ExternalInput")
    d_out = {d: nc.dram_tensor(f"out_{d}", [128, (STEPS + 1) * 4 * J], F16,
                               kind="ExternalOutput")
             for d in "fb"}

    with tile.TileContext(nc) as tc, \
         tc.tile_pool(name="wp", bufs=1) as wp, \
         tc.tile_pool(name="ap", bufs=1) as ap, \
         tc.tile_pool(name="scansb", bufs=1) as scansb, \
         tc.tile_pool(name="pp", bufs=2, space="PSUM") as pp, \
         tc.tile_pool(name="psc", bufs=2, space="PSUM") as psc:

        # ---- persistent loads ----
        def load(dram, shape, dt):
            t = wp.tile(shape, dt, tag=dram.name, name=dram.name)
            nc.sync.dma_start(t[:], dram[:])
            return t

        # order matters: srcT/wref are needed first (attention stage 1),
        # the 7MB of LSTM weights only at xg time much later
        srcT = load(d_srcT, [128, BLOC * 4 * S], F16)
        wrefT = load(d_wrefT, [128, 4 * H], F16)
        brefT = load(d_brefT, [128, 4], F32)
        id16 = load(d_id16, [128, 128], F16)
        idbf = load(d_idbf, [128, 128], BF)
        hrT = load(d_hrT, [128, BLOC * 4 * R], F16)
        hrN = load(d_hrN, [128, BLOC * 4 * H], F16)
        whh = {d: load(d_whh[d], [128, 2 * G], F16) for d in "fb"}
        bg = {d: load(d_bg[d], [128, 8], F32) for d in "fb"}
        wih = {d: load(d_wih[d], [128, 12 * G], F16) for d in "fb"}

        xg = {d: wp.tile([128, 8 * 2 * NB], F16, tag=f"xg_{d}", name=f"xg_{d}")
              for d in "fb"}
        outb = {d: wp.tile([128, (STEPS + 1) * 4 * J], F16, tag=f"outsb_{d}",
                           name=f"outsb_{d}") for d in "fb"}

        # warmup pads: forward pads blocks [0, W), backward pads [R, NB)
        for g in range(8):
            nc.vector.memset(xg["f"][:, g * 2 * NB: g * 2 * NB + 2 * W], -30.0)
            nc.vector.memset(xg["b"][:, g * 2 * NB + 2 * R: (g + 1) * 2 * NB], -30.0)

        # ---- attention + xg, per local batch ----
        for b in range(BLOC):
            hrT_b = hrT[:, b * 4 * R:(b + 1) * 4 * R]
            hrN_b = hrN[:, b * 4 * H:(b + 1) * 4 * H]

            srcT_b = srcT[:, b * 4 * S:(b + 1) * 4 * S]

            # 1) h_sT [4 Hout-tiles x S]
            hsT = ap.tile([128, 4 * S], F16, tag="tagB")
            for m in range(4):
                for sc in range(2):
                    ps = pp.tile([128, 512], F32, tag="mm")
                    for k in range(4):
                        nc.tensor.matmul(
                            ps[:],
                            wrefT[:, k * H + m * 128: k * H + (m + 1) * 128],
                            srcT_b[:, k * S + sc * 512: k * S + sc * 512 + 512],
                            start=(k == 0), stop=(k == 3))
                    nc.scalar.activation(
                        hsT[:, m * S + sc * 512: m * S + sc * 512 + 512],
                        ps[:], AF.Tanh, bias=brefT[:, m:m + 1])

            # 2) h_sN [8 S-tiles x H] = transpose(h_sT) via DMA xbar
            hsN = ap.tile([128, 8 * H], F16, tag="tagC")
            hsNv = hsN.rearrange("p (st h) -> p st h", st=8)
            for hc in range(4):
                nc.sync.dma_start_transpose(
                    hsNv[:, :, hc * 128:(hc + 1) * 128],
                    hsT[:, hc * S:(hc + 1) * S])

            # 3) eT [4 R-tiles x S] = exp(l.T), Ds partials fused into accum_out
            eT = ap.tile([128, 4 * S], BF, tag="tagD")
            ds2 = ap.tile([128, 8], F32, tag="ds2")
            for rt in range(4):
                for sc in range(2):
                    ps = pp.tile([128, 512], F32, tag="mm")
                    for k in range(4):
                        nc.tensor.matmul(
                            ps[:],
                            hrT_b[:, k * R + rt * 128: k * R + (rt + 1) * 128],
                            hsT[:, k * S + sc * 512: k * S + sc * 512 + 512],
                            start=(k == 0), stop=(k == 3))
                    nc.scalar.activation(
                        eT[:, rt * S + sc * 512: rt * S + sc * 512 + 512],
                        ps[:], AF.Exp,
                        accum_out=ds2[:, rt * 2 + sc: rt * 2 + sc + 1])

            # 4) eN [8 S-tiles x R] = transpose(eT) via DMA xbar
            eN = ap.tile([128, 8 * R], BF, tag="tagE")
            eNv = eN.rearrange("p (st r) -> p st r", st=8)
            for rc in range(4):
                nc.sync.dma_start_transpose(
                    eNv[:, :, rc * 128:(rc + 1) * 128],
                    eT[:, rc * S:(rc + 1) * S])

            # 5) softmax denominators -> scaled copies (fp16)
            dsum = ap.tile([128, 4], F32, tag="dsum")
            for rt in range(4):
                nc.vector.tensor_add(dsum[:, rt:rt + 1], ds2[:, 2 * rt:2 * rt + 1],
                                     ds2[:, 2 * rt + 1:2 * rt + 2])
            invDs = ap.tile([128, 4], F32, tag="invDs")
            nc.vector.reciprocal(invDs[:], dsum[:])
            drsum = ap.tile([128, 8], F32, tag="drsum")
            for st in range(8):
                nc.vector.tensor_reduce(
                    drsum[:, st:st + 1], eN[:, st * R:(st + 1) * R],
                    mybir.AxisListType.X, ALU.add)
            invDr = ap.tile([128, 8], F32, tag="invDr")
            nc.vector.reciprocal(invDr[:], drsum[:])

            asT = ap.tile([128, 4 * S], F16, tag="tagF")
            for rt in range(4):
                nc.vector.tensor_scalar_mul(
                    asT[:, rt * S:(rt + 1) * S], eT[:, rt * S:(rt + 1) * S],
                    invDs[:, rt:rt + 1])
            eS = ap.tile([128, 8 * R], F16, tag="tagG")
            for st in range(8):
                nc.vector.tensor_scalar_mul(
                    eS[:, st * R:(st + 1) * R], eN[:, st * R:(st + 1) * R],
                    invDr[:, st:st + 1])

            # 6) c_sT [8 S-tiles x H]  (overwrites this batch's spent srcT)
            if ALIAS_CST:
                csT = srcT[:, b * 4 * S:(b + 1) * 4 * S]
            else:
                csT = ap.tile([128, 8 * H], F16, tag="tagA")
            for st in range(8):
                ps = pp.tile([128, 512], F32, tag="mm")
                for k in range(4):
                    nc.tensor.matmul(
                        ps[:],
                        asT[:, k * S + st * 128: k * S + st * 128 + 128],
                        hrN_b[:, k * H: (k + 1) * H],
                        start=(k == 0), stop=(k == 3))
                nc.vector.tensor_copy(csT[:, st * H:(st + 1) * H], ps[:])

            # 7) c_rT [8 2H-tiles x R]  (reuses hsT slot after last hsT read)
            crT = ap.tile([128, 8 * R], F16, tag="tagB2")
            for m in range(8):
                ps = pp.tile([128, 512], F32, tag="mm")
                for k in range(8):
                    if m < 4:
                        lhsT = hsN[:, k * H + m * 128: k * H + m * 128 + 128]
                    else:
                        lhsT = csT[:, k * H + (m - 4) * 128: k * H + (m - 4) * 128 + 128]
                    nc.tensor.matmul(ps[:], lhsT, eS[:, k * R:(k + 1) * R],
                                     start=(k == 0), stop=(k == 7))
                nc.vector.tensor_copy(crT[:, m * R:(m + 1) * R], ps[:])

            # 8) xg per direction, strided into the scan's blk-major layout
            #    forward real blocks start at W, backward at 0
            for d in "fb":
                off = 2 * W if d == "f" else 0
                for g in range(8):
                    ps = pp.tile([128, 512], F32, tag="mm")
                    for k in range(12):
                        if k < 8:
                            rhs = crT[:, k * R:(k + 1) * R]
                        else:
                            rhs = hrT_b[:, (k - 8) * R:(k - 7) * R]
                        nc.tensor.matmul(
                            ps[:],
                            wih[d][:, k * G + g * 128: k * G + (g + 1) * 128],
                            rhs, start=(k == 0), stop=(k == 11))
                    dst = xg[d][:, g * 2 * NB + off + b: g * 2 * NB + off + 2 * R: 2]
                    nc.vector.tensor_scalar_add(dst, ps[:], bg[d][:, g:g + 1])

        # ---- chunked LSTM scan ----
        cst = {d: wp.tile([128, 4 * J], F32, tag=f"c_{d}", name=f"c_{d}")
               for d in "fb"}
        for d in "fb":
            nc.vector.memset(cst[d][:], 0.0)
            nc.vector.memset(outb[d][:, 0:4 * J], 0.0)

        # blk-major views of xg for the inject matmuls: [128, g, NB, 2]
        xgv = {d: xg[d].rearrange("p (g blk b) -> p g blk b", g=8, b=2)
               for d in "fb"}

        for t in range(STEPS):
            for d in "fb":
                # first block index read this step (per slot j: blk0 + j*CL)
                blk0 = t if d == "f" else (CL - 1 + W - t)
                ps = psc.tile([128, 16 * J], F32, tag=f"scps_{d}")
                hprev = outb[d][:, t * 4 * J:(t + 1) * 4 * J]
                if INJECTS_FIRST:
                    # xg injects don't depend on h -> issue them all first so
                    # the PE works while the previous step's h is in flight
                    for g in range(8):
                        nc.tensor.matmul(
                            ps[:, g * 2 * J:(g + 1) * 2 * J], id16[:],
                            xgv[d][:, g, blk0: blk0 + (J - 1) * CL + 1: CL, :],
                            start=True, stop=False)
                for g in range(8):
                    reg = ps[:, g * 2 * J:(g + 1) * 2 * J]
                    if not INJECTS_FIRST:
                        nc.tensor.matmul(
                            reg, id16[:],
                            xgv[d][:, g, blk0: blk0 + (J - 1) * CL + 1: CL, :],
                            start=True, stop=False)
                    for k in range(2):
                        nc.tensor.matmul(
                            reg,
                            whh[d][:, k * G + g * 128: k * G + (g + 1) * 128],
                            hprev[:, k * 2 * J:(k + 1) * 2 * J],
                            start=False, stop=(k == 1))
                acts = scansb.tile([128, 16 * J], F32, tag=f"acts_{d}")
                nc.scalar.activation(acts[:], ps[:], AF.Sigmoid)
                si = acts[:, 0:4 * J]
                sf = acts[:, 4 * J:8 * J]
                so = acts[:, 8 * J:12 * J]
                sg = acts[:, 12 * J:16 * J]
                cprod = scansb.tile([128, 4 * J], F32, tag=f"cprod_{d}")
                nc.vector.tensor_mul(cprod[:], cst[d][:], sf)
                t1h = scansb.tile([128, 4 * J], F32, tag=f"t1h_{d}")
                nc.vector.scalar_tensor_tensor(
                    t1h[:], sg, 0.5, si, ALU.subtract, ALU.mult)
                nc.vector.scalar_tensor_tensor(
                    cst[d][:], t1h[:], 2.0, cprod[:], ALU.mult, ALU.add)
                th = scansb.tile([128, 4 * J], F16, tag=f"th_{d}")
                nc.scalar.activation(th[:], cst[d][:], AF.Tanh)
                nc.vector.tensor_mul(
                    outb[d][:, (t + 1) * 4 * J:(t + 2) * 4 * J], th[:], so)

        for d in "fb":
            nc.sync.dma_start(d_out[d][:], outb[d][:])

    nc.compile()
    return nc


def _img_kmaj(x, p=128):
    """[K, F] -> [128, (K/128)*F] k-tile image."""
    k, f = x.shape
    return np.ascontiguousarray(
        x.reshape(k // p, p, f).transpose(1, 0, 2).reshape(p, (k // p) * f))


def _prep_core(core, inp):
    gb = [BLOC * core + i for i in range(BLOC)]
    src = np.asarray(inp["src_memory_bank"])   # [S, B, H]
    ref = np.asarray(inp["ref_memory_bank"])   # [R, B, H]

    def cat(imgs):
        return np.concatenate(imgs, axis=1)

    # g~ pre-act scale for tanh(x) = 2*sigmoid(2x)-1 (last 256 permuted rows)
    gscale = np.ones((G, 1), np.float64)
    gscale[768:] = 2.0

    m = {}
    m["srcT"] = cat([_img_kmaj(src[:, b, :].T.astype(FP16)) for b in gb])
    m["hrT"] = cat([_img_kmaj(ref[:, b, :].T.astype(FP16)) for b in gb])
    m["hrN"] = cat([_img_kmaj(ref[:, b, :].astype(FP16)) for b in gb])
    m["wrefT"] = _img_kmaj(np.asarray(inp["W_ref"]).T.astype(FP16))
    m["brefT"] = np.ascontiguousarray(
        np.asarray(inp["b_ref"]).astype(np.float32).reshape(4, 128).T)
    for d, sfx in (("f", "_f"), ("b", "_b")):
        wih = np.asarray(inp[f"W_ih{sfx}"], dtype=np.float64)[_GPERM] * gscale
        whh = np.asarray(inp[f"W_hh{sfx}"], dtype=np.float64)[_GPERM] * gscale
        m[f"wihT_{d}"] = _img_kmaj(wih.T.astype(FP16))
        m[f"whhT_{d}"] = _img_kmaj(whh.T.astype(FP16))
        bsum = ((np.asarray(inp[f"b_ih{sfx}"], dtype=np.float64)
                 + np.asarray(inp[f"b_hh{sfx}"], dtype=np.float64))[_GPERM]
                * gscale[:, 0])
        m[f"bgT_{d}"] = np.ascontiguousarray(
            bsum.astype(np.float32).reshape(8, 128).T)
    m["id16"] = np.eye(128, dtype=FP16)
    m["idbf"] = np.eye(128, dtype=BF16)
    return m


def _decode(res_list):
    """results -> [R, B, H] fp32"""
    out = np.zeros((R, B, H), dtype=np.float32)
    for c in range(N_CORES):
        for d, off in (("f", 0), ("b", HD)):
            img = np.asarray(res_list[c][f"out_{d}"])          # [128, (STEPS+1)*4J]
            x = img.reshape(128, STEPS + 1, 2, J, 2)           # p, s, k, j, b
            x = x[:, 1 + W: 1 + W + CL]                        # p, tau, k, j, b
            if d == "b":
                x = x[:, ::-1]                                 # tau' -> CL-1-tau'
            # out[j*CL + tau, b, k*128 + p]
            arr = x.transpose(3, 1, 4, 2, 0)                   # j, tau, b, k, p
            arr = np.ascontiguousarray(arr, dtype=np.float32).reshape(R, BLOC, HD)
            out[:, BLOC * c:BLOC * (c + 1), off:off + HD] = arr
    return out


def kernel(**inputs):
    if "nc" not in _CACHE:
        _CACHE["nc"] = _build_nc()
    nc = _CACHE["nc"]
    in_maps = [_prep_core(c, inputs) for c in range(N_CORES)]
    res = bass_utils.run_bass_kernel_spmd(nc, in_maps,
                                          core_ids=list(range(N_CORES)))
    return _decode(res.results)
